# revision 22
# baseline (speedup 1.0000x reference)
"""Trainium2 Bass kernel for nn_DLI_loss_2 (ragged DLI loss).

Mathematical reduction used here (verified to rel-err ~3e-7 vs the jax
reference): in the reference,

    logits[b,j,r] = hs[b,j] + xs_g[b,j,r] + fc_b
    loss_row      = logsumexp_r(logits) - logits[..., 0]

hs (the LSTM head projection) and fc_b are constant along r, so they cancel
exactly inside lse(x+c) - (x0+c).  With xs = encoder_output @ fc_w[H:] and
prefix lengths L_b = sum(mask, axis=1):

    loss_row[b,j] = log( sum_{t=j+2}^{L_b-1} exp(xs[b,t]) ) - xs[b,j+2]
    valid rows:     j+3 <= L_b  (i.e. position t=j+2 satisfies 2 <= t < L_b)
    answer = sum(valid loss_row) / count(valid)

So the whole LSTM is mathematically irrelevant; the kernel only needs
xs = x . w_x (the memory-bound part: reads all of x once), a masked exp,
a suffix-sum along t (done as a triangular matmul on the PE), a log, and a
masked reduction.

Per-core device program (pure data parallel over batch, 8 cores x 8 rows):
  - 8x: DMA x[b] [128t, 1024d] -> SBUF;
        DVE tensor_tensor_reduce: xs_col[t] = sum_d x[t,d]*wb[t,d]
  - ACT exp -> DVE mask-mult -> PE matmul with tril(ones) (suffix sums)
  - DVE add (log-safety at masked slots) -> ACT Ln -> DVE sub/mask
  - DMA out v [128, 8]; host sums all cores and divides by the count.
"""

import sys

import numpy as np

if "/opt/trn_rl_repo" not in sys.path:
    sys.path.insert(0, "/opt/trn_rl_repo")

B, T, D, H = 64, 128, 1024, 512
N_CORES = 8
BL = B // N_CORES  # batch rows per core

_CACHE = {}


def _split_multi_waits(nc):
    """This container's walrus encodes at most ONE sync-wait command per
    instruction ("Too many sync wait commands" / "ISA wrong length" in
    codegen otherwise), while Tile freely stacks several waits on one
    instruction.  Hoist all but the last wait of each instruction onto
    preceding same-engine NoOps (engine blocks on each in order, so the
    semantics are identical)."""
    import concourse.mybir as mybir

    counter = [0]
    for bb in nc.main_func.blocks:
        ins_list = bb.instructions
        out = []
        changed = False
        for ins in ins_list:
            si = getattr(ins, "sync_info", None)
            waits = list(si.on_wait) if si is not None and si.on_wait else []
            if len(waits) > 1:
                changed = True
                for w in waits[:-1]:
                    counter[0] += 1
                    out.append(
                        mybir.InstNoOp(
                            name=f"{ins.name}-wsplit{counter[0]}",
                            engine=ins.engine,
                            debug=ins.debug,
                            ins=[],
                            outs=[],
                            sync_info=mybir.SyncInfo(on_wait=[w], on_update=[]),
                        )
                    )
                ins.sync_info = mybir.SyncInfo(
                    on_wait=[waits[-1]], on_update=list(si.on_update or [])
                )
            out.append(ins)
        if changed:
            bb.instructions = out


def _build_bass():
    import concourse.bass as bass
    import concourse.mybir as mybir
    from concourse.tile import TileContext

    f32 = mybir.dt.float32
    bf16 = mybir.dt.bfloat16
    nc = bass.Bass(trn_type="TRN2")

    x_d = nc.dram_tensor("x", [BL, T, D], bf16, kind="ExternalInput")
    wb_d = nc.dram_tensor("wb", [128, D], bf16, kind="ExternalInput")
    rmT_d = nc.dram_tensor("rmT", [T, BL], f32, kind="ExternalInput")
    rmTe_d = nc.dram_tensor("rmTe", [T, BL], f32, kind="ExternalInput")
    v_d = nc.dram_tensor("v", [T, BL], f32, kind="ExternalOutput")
    tri_np = np.tril(np.ones((T, T), dtype=np.float32))  # tri[u,t]=1 iff u>=t
    tri_d = nc.inline_tensor(tri_np, name="tri")

    with TileContext(nc) as tc:
        with (
            tc.tile_pool(name="consts", bufs=1) as consts,
            tc.tile_pool(name="xp", bufs=1) as xp,
            tc.tile_pool(name="scratch", bufs=2) as scratch,
            tc.tile_pool(name="small", bufs=1) as small,
            tc.psum_pool(name="psum", bufs=1) as psum,
        ):
            # wb on sync, x0 on scalar concurrently (first DVE op then waits
            # only two DMA sems); remaining tiles alternate queues as whole
            # transfers (HWDGE triggers cost ~640ns each, so fewer+bigger
            # transfers keep HBM saturated)
            wb_t = consts.tile([128, D], bf16)
            nc.sync.dma_start(out=wb_t[:], in_=wb_d[:, :])
            xts = []
            for b in range(BL):
                xt = xp.tile([T, D], bf16, name=f"xt{b}")
                if b % 2 == 0:
                    nc.scalar.dma_start(out=xt[:], in_=x_d[b, :, :])
                else:
                    nc.sync.dma_start(out=xt[:], in_=x_d[b, :, :])
                xts.append(xt)

            # epilogue consts on the SWDGE (gpsimd) path - independent queue,
            # not needed until the suffix-sum matmul.  rmTe = rmask + 1e-30:
            # keeps every suffix sum strictly positive so Ln is finite at
            # masked slots (then exactly zeroed by the pure rmask multiply),
            # letting Ln read the matmul PSUM directly with no bias-add op.
            rmT_t = consts.tile([T, BL], f32)
            nc.gpsimd.dma_start(out=rmT_t[:], in_=rmT_d[:, :])
            rmTe_t = consts.tile([T, BL], f32)
            nc.gpsimd.dma_start(out=rmTe_t[:], in_=rmTe_d[:, :])
            tri_t = consts.tile([T, T], f32)
            nc.gpsimd.dma_start(out=tri_t[:], in_=tri_d[:, :])

            # Per tile, two ways to get xs_col = sum_d x*wb, measured:
            #  - fused scalar_tensor_tensor: 1.22us on DVE (1x uop only)
            #  - bf16 tensor_mul (0.69us DVE, 2x_1P) + ACTIVATE Copy with
            #    accum_out (1.13us on ScalarE)
            # Alternate them so DVE and ScalarE overlap: DVE ~7.6us,
            # ScalarE ~4.5us instead of 9.8us serial on DVE alone.
            # b even -> TT+ACT (ScalarE reduce starts early), b odd -> fused
            # STT, so the last tile (b=7) finishes on DVE while ScalarE's
            # final reduce (b=6) completes in parallel.
            xs_cols = small.tile([T, BL], f32)
            for b in range(BL):
                prod = scratch.tile([T, D], bf16)
                if b % 2 == 1:
                    nc.vector.scalar_tensor_tensor(
                        out=prod[:],
                        in0=xts[b][:],
                        scalar=1.0,
                        in1=wb_t[:],
                        op0=mybir.AluOpType.mult,
                        op1=mybir.AluOpType.mult,
                        accum_out=xs_cols[:, b : b + 1],
                    )
                else:
                    nc.vector.tensor_mul(prod[:], xts[b][:], wb_t[:])
                    acopy = scratch.tile([T, D], bf16, name="acopy")
                    nc.scalar.activation(
                        acopy[:],
                        prod[:],
                        mybir.ActivationFunctionType.Copy,
                        accum_out=xs_cols[:, b : b + 1],
                    )

            ex = small.tile([T, BL], f32)
            nc.scalar.activation(ex[:], xs_cols[:], mybir.ActivationFunctionType.Exp)
            ee = small.tile([T, BL], f32)
            nc.vector.tensor_mul(ee[:], ex[:], rmTe_t[:])
            sst_ps = psum.tile([T, BL], f32)
            nc.tensor.matmul(sst_ps[:], lhsT=tri_t[:], rhs=ee[:], start=True, stop=True)
            logss = small.tile([T, BL], f32)
            nc.scalar.activation(logss[:], sst_ps[:], mybir.ActivationFunctionType.Ln)
            diff = small.tile([T, BL], f32)
            nc.vector.tensor_sub(diff[:], logss[:], xs_cols[:])
            vv = small.tile([T, BL], f32)
            nc.vector.tensor_mul(vv[:], diff[:], rmT_t[:])
            nc.sync.dma_start(out=v_d[:, :], in_=vv[:])

    _split_multi_waits(nc)
    return nc


def _get_nc():
    if "nc" not in _CACHE:
        _CACHE["nc"] = _build_bass()
    return _CACHE["nc"]


def kernel(encoder_output, mask, W_ih, W_hh, b_ih, b_hh, fc_w, fc_b, _perf=None):
    from concourse.bass_utils import run_bass_kernel_spmd

    x = np.ascontiguousarray(np.asarray(encoder_output, dtype=np.float32))
    m = np.asarray(mask)
    w_x = np.asarray(fc_w, dtype=np.float32)[H:]
    assert x.shape == (B, T, D) and m.shape == (B, T) and w_x.shape == (D,)

    lengths = m.astype(np.int64).sum(axis=1)  # reference uses only sum(mask)
    t_idx = np.arange(T)
    rmask = ((t_idx[None, :] >= 2) & (t_idx[None, :] < lengths[:, None])).astype(
        np.float32
    )  # [B, T] row-validity: position t=j+2 of a valid row j
    cnt = float(rmask.sum())

    import ml_dtypes

    bf16 = ml_dtypes.bfloat16
    xb = x.astype(bf16)
    wb = np.ascontiguousarray(np.broadcast_to(w_x.astype(bf16), (128, D)))
    in_maps = []
    for c in range(N_CORES):
        sl = slice(c * BL, (c + 1) * BL)
        rmT = np.ascontiguousarray(rmask[sl].T)  # [T, BL]
        in_maps.append(
            {
                "x": np.ascontiguousarray(xb[sl]),
                "wb": wb,
                "rmT": rmT,
                "rmTe": np.ascontiguousarray(rmT + np.float32(1e-30)),
            }
        )

    nc = _get_nc()
    kw = dict(_perf or {})
    br = run_bass_kernel_spmd(nc, in_maps, core_ids=list(range(N_CORES)), **kw)
    if _perf is not None:
        _CACHE["last_result"] = br

    total = 0.0
    for r in br.results:
        total += float(r["v"].astype(np.float64).sum())
    if cnt <= 0:
        return np.zeros((), dtype=np.float32)
    return np.asarray(np.float32(total / cnt))


# revision 23
# speedup vs baseline: 1.0540x; 1.0540x over previous
"""Trainium2 Bass kernel for nn_DLI_loss_2 (ragged DLI loss).

Mathematical reduction used here (verified to rel-err ~3e-7 vs the jax
reference): in the reference,

    logits[b,j,r] = hs[b,j] + xs_g[b,j,r] + fc_b
    loss_row      = logsumexp_r(logits) - logits[..., 0]

hs (the LSTM head projection) and fc_b are constant along r, so they cancel
exactly inside lse(x+c) - (x0+c).  With xs = encoder_output @ fc_w[H:] and
prefix lengths L_b = sum(mask, axis=1):

    loss_row[b,j] = log( sum_{t=j+2}^{L_b-1} exp(xs[b,t]) ) - xs[b,j+2]
    valid rows:     j+3 <= L_b  (i.e. position t=j+2 satisfies 2 <= t < L_b)
    answer = sum(valid loss_row) / count(valid)

So the whole LSTM is mathematically irrelevant; the kernel only needs
xs = x . w_x (the memory-bound part: reads all of x once), a masked exp,
a suffix-sum along t (done as a triangular matmul on the PE), a log, and a
masked reduction.

Per-core device program (pure data parallel over batch, 8 cores x 8 rows):
  - 8x: DMA x[b] [128t, 1024d] -> SBUF;
        DVE tensor_tensor_reduce: xs_col[t] = sum_d x[t,d]*wb[t,d]
  - ACT exp -> DVE mask-mult -> PE matmul with tril(ones) (suffix sums)
  - DVE add (log-safety at masked slots) -> ACT Ln -> DVE sub/mask
  - DMA out v [128, 8]; host sums all cores and divides by the count.
"""

import sys

import numpy as np

if "/opt/trn_rl_repo" not in sys.path:
    sys.path.insert(0, "/opt/trn_rl_repo")

B, T, D, H = 64, 128, 1024, 512
N_CORES = 8
BL = B // N_CORES  # batch rows per core

_CACHE = {}


def _split_multi_waits(nc):
    """This container's walrus encodes at most ONE sync-wait command per
    instruction ("Too many sync wait commands" / "ISA wrong length" in
    codegen otherwise), while Tile freely stacks several waits on one
    instruction.  Hoist all but the last wait of each instruction onto
    preceding same-engine NoOps (engine blocks on each in order, so the
    semantics are identical)."""
    import concourse.mybir as mybir

    counter = [0]
    for bb in nc.main_func.blocks:
        ins_list = bb.instructions
        out = []
        changed = False
        for ins in ins_list:
            si = getattr(ins, "sync_info", None)
            waits = list(si.on_wait) if si is not None and si.on_wait else []
            if len(waits) > 1:
                changed = True
                for w in waits[:-1]:
                    counter[0] += 1
                    out.append(
                        mybir.InstNoOp(
                            name=f"{ins.name}-wsplit{counter[0]}",
                            engine=ins.engine,
                            debug=ins.debug,
                            ins=[],
                            outs=[],
                            sync_info=mybir.SyncInfo(on_wait=[w], on_update=[]),
                        )
                    )
                ins.sync_info = mybir.SyncInfo(
                    on_wait=[waits[-1]], on_update=list(si.on_update or [])
                )
            out.append(ins)
        if changed:
            bb.instructions = out


def _build_bass():
    import concourse.bass as bass
    import concourse.mybir as mybir
    from concourse.tile import TileContext

    f32 = mybir.dt.float32
    bf16 = mybir.dt.bfloat16
    nc = bass.Bass(trn_type="TRN2")

    x_d = nc.dram_tensor("x", [BL, T, D], bf16, kind="ExternalInput")
    wb_d = nc.dram_tensor("wb", [128, D], bf16, kind="ExternalInput")
    rmT_d = nc.dram_tensor("rmT", [T, BL], f32, kind="ExternalInput")
    rmTe_d = nc.dram_tensor("rmTe", [T, BL], f32, kind="ExternalInput")
    v_d = nc.dram_tensor("v", [T, BL], f32, kind="ExternalOutput")
    tri_np = np.tril(np.ones((T, T), dtype=np.float32))  # tri[u,t]=1 iff u>=t
    tri_d = nc.inline_tensor(tri_np, name="tri")

    with TileContext(nc) as tc:
        with (
            tc.tile_pool(name="consts", bufs=1) as consts,
            tc.tile_pool(name="xp", bufs=1) as xp,
            tc.tile_pool(name="scratch", bufs=2) as scratch,
            tc.tile_pool(name="small", bufs=1) as small,
            tc.psum_pool(name="psum", bufs=1) as psum,
        ):
            # DMA sizing tradeoff (measured): every DMA completion semaphore
            # carries a ~1.5us straggler, so the compute chain is paced by
            # per-queue completion cadence, not raw bandwidth.  Small first
            # transfers start the chain early; big paired transfers amortize
            # the straggler for steady state.
            wb_t = consts.tile([128, D], bf16)
            nc.sync.dma_start(out=wb_t[:, : D // 2], in_=wb_d[:, : D // 2])
            nc.scalar.dma_start(out=wb_t[:, D // 2 :], in_=wb_d[:, D // 2 :])

            xts = []
            for b in range(BL):
                xts.append(xp.tile([T, D], bf16, name=f"xt{b}"))
            # x0 halves (earliest possible first-compute), x1 whole,
            # then 512KB pairs: (2,3)+(6,7) on scalar, (4,5) on sync
            nc.sync.dma_start(out=xts[0][:, : D // 2], in_=x_d[0, :, : D // 2])
            nc.scalar.dma_start(out=xts[0][:, D // 2 :], in_=x_d[0, :, D // 2 :])
            nc.sync.dma_start(out=xts[1][:], in_=x_d[1, :, :])
            for b0, eng in ((2, nc.scalar), (4, nc.sync), (6, nc.scalar)):
                pair = xp.tile([T, 2, D], bf16, name=f"xpair{b0}")
                eng.dma_start(
                    out=pair[:],
                    in_=x_d[b0 : b0 + 2, :, :].rearrange("b t d -> t b d"),
                )
                xts[b0] = pair[:, 0, :]
                xts[b0 + 1] = pair[:, 1, :]

            # epilogue consts on the SWDGE (gpsimd) path - independent queue,
            # not needed until the suffix-sum matmul.  rmTe = rmask + 1e-30:
            # keeps every suffix sum strictly positive so Ln is finite at
            # masked slots (then exactly zeroed by the pure rmask multiply),
            # letting Ln read the matmul PSUM directly with no bias-add op.
            rmT_t = consts.tile([T, BL], f32)
            nc.gpsimd.dma_start(out=rmT_t[:], in_=rmT_d[:, :])
            rmTe_t = consts.tile([T, BL], f32)
            nc.gpsimd.dma_start(out=rmTe_t[:], in_=rmTe_d[:, :])
            tri_t = consts.tile([T, T], f32)
            nc.gpsimd.dma_start(out=tri_t[:], in_=tri_d[:, :])

            # Per tile, two ways to get xs_col = sum_d x*wb, measured:
            #  - fused scalar_tensor_tensor: 1.22us on DVE (1x uop only)
            #  - bf16 tensor_mul (0.69us DVE, 2x_1P) + ACTIVATE Copy with
            #    accum_out (1.13us on ScalarE)
            # Alternate them so DVE and ScalarE overlap: DVE ~7.6us,
            # ScalarE ~4.5us instead of 9.8us serial on DVE alone.
            # b even -> TT+ACT (ScalarE reduce starts early), b odd -> fused
            # STT, so the last tile (b=7) finishes on DVE while ScalarE's
            # final reduce (b=6) completes in parallel.
            xs_cols = small.tile([T, BL], f32)
            for b in range(BL):
                prod = scratch.tile([T, D], bf16)
                if b % 2 == 1:
                    nc.vector.scalar_tensor_tensor(
                        out=prod[:],
                        in0=xts[b][:],
                        scalar=1.0,
                        in1=wb_t[:],
                        op0=mybir.AluOpType.mult,
                        op1=mybir.AluOpType.mult,
                        accum_out=xs_cols[:, b : b + 1],
                    )
                else:
                    nc.vector.tensor_mul(prod[:], xts[b][:], wb_t[:])
                    acopy = scratch.tile([T, D], bf16, name="acopy")
                    nc.scalar.activation(
                        acopy[:],
                        prod[:],
                        mybir.ActivationFunctionType.Copy,
                        accum_out=xs_cols[:, b : b + 1],
                    )

            ex = small.tile([T, BL], f32)
            nc.scalar.activation(ex[:], xs_cols[:], mybir.ActivationFunctionType.Exp)
            ee = small.tile([T, BL], f32)
            nc.vector.tensor_mul(ee[:], ex[:], rmTe_t[:])
            sst_ps = psum.tile([T, BL], f32)
            nc.tensor.matmul(sst_ps[:], lhsT=tri_t[:], rhs=ee[:], start=True, stop=True)
            logss = small.tile([T, BL], f32)
            nc.scalar.activation(logss[:], sst_ps[:], mybir.ActivationFunctionType.Ln)
            diff = small.tile([T, BL], f32)
            nc.vector.tensor_sub(diff[:], logss[:], xs_cols[:])
            vv = small.tile([T, BL], f32)
            nc.vector.tensor_mul(vv[:], diff[:], rmT_t[:])
            nc.sync.dma_start(out=v_d[:, :], in_=vv[:])

    _split_multi_waits(nc)
    return nc


def _get_nc():
    if "nc" not in _CACHE:
        _CACHE["nc"] = _build_bass()
    return _CACHE["nc"]


def kernel(encoder_output, mask, W_ih, W_hh, b_ih, b_hh, fc_w, fc_b, _perf=None):
    from concourse.bass_utils import run_bass_kernel_spmd

    x = np.ascontiguousarray(np.asarray(encoder_output, dtype=np.float32))
    m = np.asarray(mask)
    w_x = np.asarray(fc_w, dtype=np.float32)[H:]
    assert x.shape == (B, T, D) and m.shape == (B, T) and w_x.shape == (D,)

    lengths = m.astype(np.int64).sum(axis=1)  # reference uses only sum(mask)
    t_idx = np.arange(T)
    rmask = ((t_idx[None, :] >= 2) & (t_idx[None, :] < lengths[:, None])).astype(
        np.float32
    )  # [B, T] row-validity: position t=j+2 of a valid row j
    cnt = float(rmask.sum())

    import ml_dtypes

    bf16 = ml_dtypes.bfloat16
    xb = x.astype(bf16)
    wb = np.ascontiguousarray(np.broadcast_to(w_x.astype(bf16), (128, D)))
    in_maps = []
    for c in range(N_CORES):
        sl = slice(c * BL, (c + 1) * BL)
        rmT = np.ascontiguousarray(rmask[sl].T)  # [T, BL]
        in_maps.append(
            {
                "x": np.ascontiguousarray(xb[sl]),
                "wb": wb,
                "rmT": rmT,
                "rmTe": np.ascontiguousarray(rmT + np.float32(1e-30)),
            }
        )

    nc = _get_nc()
    kw = dict(_perf or {})
    br = run_bass_kernel_spmd(nc, in_maps, core_ids=list(range(N_CORES)), **kw)
    if _perf is not None:
        _CACHE["last_result"] = br

    total = 0.0
    for r in br.results:
        total += float(r["v"].astype(np.float64).sum())
    if cnt <= 0:
        return np.zeros((), dtype=np.float32)
    return np.asarray(np.float32(total / cnt))
